# revision 19
# baseline (speedup 1.0000x reference)
"""Trainium2 Bass kernel for nn_ChimeraNet (encoder -> 10-step Euler RNN -> LN -> readout).

Data-parallel over 8 NeuronCores: each core gets 1024 rows of the batch and a
replicated set of (host-prefolded) weights.

Math (per core, R=1024 rows, D=1024):
    drive_in = x @ W_c + bias          with W_c = W_enc.T @ W_in (host-folded)
    h_{t+1}  = 0.8 h_t + 0.2 tanh(h_t @ W_res + drive_in),  h_0 = 0, 10 steps
    out      = inv*(h @ W2.T) + (-mu*inv)*w1 + b2           (LayerNorm folded into readout)
  where mu/var are LayerNorm stats over D, inv = rsqrt(var+eps),
  W2 = W_out * ln_g,  w1 = W2 @ 1,  b2 = W_out @ ln_b + b_out.

The integration state is stored scaled, u_t = h_t / 0.2, with 0.2 folded into
W_res and the readout weights on the host.  The recurrence becomes
    u_{t+1} = 0.8*u_t + tanh(u_t @ (0.2 W_res) + drive_in)
so the state update is a single fused scalar_tensor_tensor DVE op per tile.

All matmul operands are fp16 (same 1 cycle/row PE rate as fp32r at N=512, but
half the DMA volume, half the SBUF traffic and cheap LDWEIGHTS); PSUM
accumulation and the LayerNorm statistics stay fp32.  x is transposed during
load by the DGE XBAR (dma_start_transpose), so the PE does no transposes in
the head.  The encoder runs k-outer over 7 concurrent PSUM banks so it
consumes W_c tiles as they arrive from HBM.  In the Euler loop each weight
block is loaded once and used for both 512-row slices of the moving dim.  The
output is staged as one [128, 8, 10] SBUF tile and shipped with a single
contiguous DMA; the host undoes the row interleave.
"""

import os
import sys

import numpy as np

try:
    import concourse.bass as bass  # noqa: F401
except ImportError:  # pragma: no cover - fresh grading env without PYTHONPATH
    for p in ("/root/.axon_site", "/root/.axon_site/_ro/trn_rl_repo",
              "/root/.axon_site/_ro/pypackages", "/opt/trn_rl_repo"):
        if os.path.isdir(p) and p not in sys.path:
            sys.path.append(p)
    import concourse.bass as bass

from contextlib import ExitStack

import concourse.tile as tile
from concourse import bacc, bass_utils, mybir
from concourse.masks import make_identity

N_CORES = 8
B = 8192
R = B // N_CORES        # rows per core
D = 1024                # latent dim
KX = 784                # encoder input dim
KXP = 896               # padded to 7*128 for the XBAR transpose
DT_STEP = 0.2
STEPS = 10
EPS = 1e-5

F16 = mybir.dt.float16
F32 = mybir.dt.float32
AF = mybir.ActivationFunctionType
ALU = mybir.AluOpType

KD = D // 128           # 8 k/m tiles over D
NS = R // 512           # 2 moving-dim slices of 512
KE = KXP // 128         # 7 encoder k tiles
KW_LAST = KX - 6 * 128  # 16 real rows in the last encoder k tile
NWARM0 = 5              # dependency-free PE warmups before the encoder
NWARM_K = 1             # warmups between encoder k groups (HAM keep-alive)


def _build_program():
    nc = bacc.Bacc("TRN2", target_bir_lowering=False, debug=False)

    # all tensors arrive host-packed partition-major so every DMA moves long
    # contiguous per-partition runs (128 fat descriptors, no packet storms)
    x = nc.dram_tensor("x", [128, NS, KE, 512], F16, kind="ExternalInput").ap()
    w_c = nc.dram_tensor("w_c", [128, KE, D], F16, kind="ExternalInput").ap()
    w_res = nc.dram_tensor("w_res", [128, KD, D], F16, kind="ExternalInput").ap()
    bias = nc.dram_tensor("bias", [128, KD], F32, kind="ExternalInput").ap()
    w2a = nc.dram_tensor("w2a", [128, KD, 11], F16, kind="ExternalInput").ap()
    w1 = nc.dram_tensor("w1", [10], F32, kind="ExternalInput").ap()
    b2 = nc.dram_tensor("b2", [10], F32, kind="ExternalInput").ap()
    out = nc.dram_tensor("out", [128, KD, 10], F16, kind="ExternalOutput").ap()

    with tile.TileContext(nc) as tc, ExitStack() as ctx:
        state = ctx.enter_context(tc.tile_pool(name="state", bufs=1))
        consts = ctx.enter_context(tc.tile_pool(name="consts", bufs=1))

        # ---- persistent SBUF tensors -------------------------------------
        xt = state.tile([128, NS, KE, 512], F16, name="xt")       # x.T
        wc_big = state.tile([128, KE, D], F16, name="wc")
        wres_big = state.tile([128, KD, D], F16, name="wr")
        drive = [state.tile([128, R], F16, name=f"dr{m}", tag=f"dr{m}")
                 for m in range(KD)]
        g = [[state.tile([128, R], F16, name=f"g{b}_{m}", tag=f"g{b}_{m}")
              for m in range(KD)] for b in range(2)]
        sq_tiles = [[state.tile([128, 512], F16, name=f"sq{n}_{m}", tag=f"sq{n}_{m}")
                     for m in range(KD)] for n in range(NS)]

        # ---- input DMAs (issue order = priority within each queue) ------
        # sync (HWDGE): x XBAR-transposes first (needed earliest), then the
        # tail of W_c, then half of W_res.  scalar (HWDGE): head of W_c then
        # the other half of W_res.  gpsimd (SWDGE): small constants.
        def dma_wc(eng, k):
            kw = 128 if k < 6 else KW_LAST
            eng.dma_start(out=wc_big[:kw, k, :], in_=w_c[:kw, k, :])

        def dma_xt(eng, h, k):
            kw = 128 if k < 6 else KW_LAST
            eng.dma_start(out=xt[:kw, h, k, :], in_=x[:kw, h, k, :])

        # need-order interleave across the two HWDGE queues: the encoder
        # consumes (wc[k], xt[h0,k]) k-ascending, then xt[h1,k], then wres[k].
        dma_wc(nc.sync, 0)
        for k in (0, 1, 2):
            dma_xt(nc.sync, 0, k)
        dma_wc(nc.sync, 3)
        for k in (3, 4):
            dma_xt(nc.sync, 0, k)
        dma_wc(nc.sync, 5)
        for k in (5, 6):
            dma_xt(nc.sync, 0, k)
        for k in range(KE):
            dma_xt(nc.sync, 1, k)
        for k in (6, 7):
            nc.sync.dma_start(out=wres_big[:, k, :], in_=w_res[:, k, :])
        for k in (1, 2, 4, 6):
            dma_wc(nc.scalar, k)
        for k in range(6):
            nc.scalar.dma_start(out=wres_big[:, k, :], in_=w_res[:, k, :])

        warm_src = consts.tile([128, 512], F16)
        nc.gpsimd.memset(warm_src, 0.01)
        warm_keep = consts.tile([128, 1], F32)
        ident = consts.tile([128, 128], F16)
        make_identity(nc, ident)
        bias_sb = consts.tile([128, KD], F32)
        nc.gpsimd.dma_start(out=bias_sb, in_=bias)
        w2a_sb = consts.tile([128, KD, 11], F16)
        nc.gpsimd.dma_start(out=w2a_sb, in_=w2a)
        w1_bc = consts.tile([128, 10], F32)
        nc.gpsimd.dma_start(out=w1_bc, in_=bass.AP(tensor=w1.tensor, offset=w1.offset,
                                                   ap=[[0, 128]] + list(w1.ap)))
        b2_bc = consts.tile([128, 10], F32)
        nc.gpsimd.dma_start(out=b2_bc, in_=bass.AP(tensor=b2.tensor, offset=b2.offset,
                                                   ap=[[0, 128]] + list(b2.ap)))
        ones_sb = consts.tile([128, 1], F16)
        nc.vector.memset(ones_sb, 1.0)
        eps_sb = consts.tile([128, 1], F32)
        nc.vector.memset(eps_sb, EPS)

        def enc_drain2(m, sl, ps):
            """psum -> drive (bias add) and step-0 state u1 = tanh(drive)."""
            nc.scalar.activation(drive[m][:, sl], ps, AF.Identity,
                                 bias=bias_sb[:, m:m + 1], scale=1.0)
            nc.scalar.activation(g[0][m][:, sl], drive[m][:, sl], AF.Tanh)

        # ---- encoder ------------------------------------------------------
        with ExitStack() as encctx:
            wrm = encctx.enter_context(tc.tile_pool(name="wrm", bufs=1, space="PSUM"))
            encp = encctx.enter_context(tc.tile_pool(name="encp", bufs=1, space="PSUM"))

            last_warm = [None]

            def warm(n, tag):
                for w in range(n):
                    wp = wrm.tile([128, 512], F32, name=f"warm_{tag}_{w}", tag="w")
                    nc.tensor.matmul(wp, lhsT=warm_src[:, :128], rhs=warm_src,
                                     start=True, stop=True)
                    last_warm[0] = wp

            warm(NWARM0, "pre")

            # n=0 slice: k-outer over 7 concurrent psum banks (m=0..6), so
            # each W_c tile is consumed ~as it lands from HBM.
            sl0 = slice(0, 512)
            enc_ps = [encp.tile([128, 512], F32, name=f"eps{m}", tag=f"e{m}")
                      for m in range(KD - 1)]
            for k in range(KE):
                kw = 128 if k < 6 else KW_LAST
                for m in range(KD - 1):
                    nc.tensor.matmul(enc_ps[m],
                                     lhsT=wc_big[:kw, k, m * 128:(m + 1) * 128],
                                     rhs=xt[:kw, 0, k, :],
                                     start=(k == 0), stop=(k == KE - 1))
                if k < KE - 1:
                    warm(NWARM_K, f"k{k}")
            for m in range(KD - 1):
                enc_drain2(m, sl0, enc_ps[m])

            # remaining groups k-inner: (m=7, n=0) then all m for n=1
            rest = [(KD - 1, 0)] + [(m, 1) for m in range(KD)]
            for i, (m, n) in enumerate(rest):
                sl = slice(n * 512, (n + 1) * 512)
                ps = encp.tile([128, 512], F32, name=f"ep2_{m}_{n}", tag=f"e{i % 7}")
                for k in range(KE):
                    kw = 128 if k < 6 else KW_LAST
                    nc.tensor.matmul(ps,
                                     lhsT=wc_big[:kw, k, m * 128:(m + 1) * 128],
                                     rhs=xt[:kw, n, k, :],
                                     start=(k == 0), stop=(k == KE - 1))
                enc_drain2(m, sl, ps)
            nc.vector.tensor_copy(warm_keep, last_warm[0][:, :1])

        # ---- Euler loop + readout ----------------------------------------
        psum = ctx.enter_context(tc.tile_pool(name="mm", bufs=6, space="PSUM"))
        stp = ctx.enter_context(tc.tile_pool(name="st", bufs=2, space="PSUM"))
        tmp = ctx.enter_context(tc.tile_pool(name="tmp", bufs=3))
        tail = ctx.enter_context(tc.tile_pool(name="tail", bufs=1))

        for s in range(1, STEPS):
            cur, nxt = g[(s + 1) % 2], g[s % 2]
            for m in range(KD):
                msl = slice(m * 128, (m + 1) * 128)
                ps = [psum.tile([128, 512], F32, name=f"ps{s}_{m}_{n}", tag="mm")
                      for n in range(NS)]
                for k in range(KD):
                    for n in range(NS):
                        nc.tensor.matmul(ps[n], lhsT=wres_big[:, k, msl],
                                         rhs=cur[k][:, n * 512:(n + 1) * 512],
                                         start=(k == 0), stop=(k == KD - 1))
                for n in range(NS):
                    sl = slice(n * 512, (n + 1) * 512)
                    d = tmp.tile([128, 512], F32, name=f"d{s}_{m}_{n}", tag="d")
                    nc.vector.tensor_add(d, ps[n], drive[m][:, sl])
                    t = tmp.tile([128, 512], F16, name=f"t{s}_{m}_{n}", tag="t")
                    nc.scalar.activation(t, d, AF.Tanh)
                    # u' = 0.8*u + t  (single fused op, all-fp16)
                    nc.vector.scalar_tensor_tensor(
                        nxt[m][:, sl], in0=cur[m][:, sl], scalar=1.0 - DT_STEP,
                        in1=t, op0=ALU.mult, op1=ALU.add)
                    if s == STEPS - 1:
                        nc.vector.tensor_mul(sq_tiles[n][m], nxt[m][:, sl],
                                             nxt[m][:, sl])

        gfin = g[(STEPS - 1) % 2]

        # ---- tail: LN stats + readout ------------------------------------
        y_sb = tail.tile([11, R], F16)
        s2_sb = tail.tile([1, R], F16)
        o_all = tail.tile([128, KD, 10], F16)

        def stat_chain_batch(n):
            """LN stats + readout combine for row tiles rt=4n..4n+3, batched."""
            tp = stp.tile([128, 4, 14], F16, name=f"tp{n}", tag="st")
            for j in range(4):
                sl = slice((n * 4 + j) * 128, (n * 4 + j + 1) * 128)
                nc.tensor.transpose(tp[:, j, 0:11], y_sb[:, sl], ident[:11, :11])
                nc.tensor.transpose(tp[:, j, 12:13], s2_sb[:, sl], ident[:1, :1])
            mu4 = tail.tile([128, 4], F32, name=f"mu{n}", tag="mu", bufs=2)
            nc.scalar.mul(mu4, tp[:, :, 10], -DT_STEP / D)        # -mean(h)
            ex4 = tail.tile([128, 4], F32, name=f"ex{n}", tag="ex", bufs=2)
            nc.scalar.mul(ex4, tp[:, :, 12], DT_STEP * DT_STEP / D)  # E[h^2]
            var4 = tail.tile([128, 4], F32, name=f"var{n}", tag="var", bufs=2)
            # var = E[h^2] - mu^2 = -(mu4*mu4) + ex4
            nc.vector.scalar_tensor_tensor(var4, in0=mu4, scalar=-1.0,
                                           op0=ALU.mult, in1=mu4, op1=ALU.mult)
            nc.vector.tensor_add(var4, var4, ex4)
            sd4 = tail.tile([128, 4], F32, name=f"sd{n}", tag="sd", bufs=2)
            nc.scalar.activation(sd4, var4, AF.Sqrt, bias=eps_sb, scale=1.0)
            inv4 = tail.tile([128, 4], F32, name=f"inv{n}", tag="inv", bufs=2)
            nc.vector.reciprocal(inv4, sd4)
            qn4 = tail.tile([128, 4], F32, name=f"qn{n}", tag="qn", bufs=2)
            nc.vector.tensor_mul(qn4, mu4, inv4)                  # -mu*inv
            t1 = tail.tile([128, 4, 10], F32, name=f"t1_{n}", tag="t1", bufs=2)
            t2 = tail.tile([128, 4, 10], F32, name=f"t2_{n}", tag="t2", bufs=2)
            for j in range(4):
                nc.vector.tensor_scalar_mul(t1[:, j, :], tp[:, j, 0:10],
                                            inv4[:, j:j + 1])
                nc.vector.scalar_tensor_tensor(t2[:, j, :], in0=w1_bc,
                                               scalar=qn4[:, j:j + 1],
                                               in1=t1[:, j, :],
                                               op0=ALU.mult, op1=ALU.add)
                nc.vector.tensor_add(o_all[:, n * 4 + j, :], t2[:, j, :], b2_bc)

        for n in range(NS):
            sl = slice(n * 512, (n + 1) * 512)
            yp = psum.tile([11, 512], F32, name=f"yp{n}", tag="mm")
            for k in range(KD):
                nc.tensor.matmul(yp, lhsT=w2a_sb[:, k, :], rhs=gfin[k][:, sl],
                                 start=(k == 0), stop=(k == KD - 1))
            nc.scalar.copy(y_sb[:, sl], yp)
            s2 = psum.tile([1, 512], F32, name=f"s2p{n}", tag="mm")
            for k in range(KD):
                nc.tensor.matmul(s2, lhsT=ones_sb, rhs=sq_tiles[n][k],
                                 start=(k == 0), stop=(k == KD - 1))
            nc.scalar.copy(s2_sb[:, sl], s2)
            stat_chain_batch(n)
            nc.sync.dma_start(out=out[:, n * 4:(n + 1) * 4, :],
                              in_=o_all[:, n * 4:(n + 1) * 4, :])

    nc.compile()
    return nc


_NC_CACHE = None


def _get_program():
    global _NC_CACHE
    if _NC_CACHE is None:
        _NC_CACHE = _build_program()
    return _NC_CACHE


def _prepare_in_maps(inputs):
    x = np.asarray(inputs["x"], dtype=np.float32)
    w_enc = np.asarray(inputs["W_enc"], dtype=np.float32)
    w_res = np.asarray(inputs["W_res"], dtype=np.float32)
    w_in = np.asarray(inputs["W_in"], dtype=np.float32)
    bias = np.asarray(inputs["bias"], dtype=np.float32)
    ln_g = np.asarray(inputs["ln_g"], dtype=np.float32)
    ln_b = np.asarray(inputs["ln_b"], dtype=np.float32)
    w_out = np.asarray(inputs["W_out"], dtype=np.float32)
    b_out = np.asarray(inputs["b_out"], dtype=np.float32)

    w_c = (w_enc.T.astype(np.float64) @ w_in.astype(np.float64)).astype(np.float32)
    w2 = w_out * ln_g[None, :]                       # [10, D]
    # state is u = h/0.2: fold 0.2 into W_res (matmul input) and readout/stats
    w_res_s = (DT_STEP * w_res.astype(np.float64)).astype(np.float32)
    w2a = np.empty((D, 11), np.float32)
    w2a[:, :10] = DT_STEP * w2.T                     # readout: gives W2 @ h.T
    w2a[:, 10] = 1.0                                 # S1 column: sum_D u
    w1v = w2.sum(axis=1).astype(np.float32)
    b2v = (w_out.astype(np.float64) @ ln_b.astype(np.float64)
           + b_out.astype(np.float64)).astype(np.float32)

    # pack everything partition-major ([128, ...] with long contiguous rows)
    wc16 = np.zeros((KXP, D), np.float16)
    wc16[:KX] = w_c.astype(np.float16)
    wc_pk = wc16.reshape(KE, 128, D).transpose(1, 0, 2)
    wres_pk = w_res_s.astype(np.float16).reshape(KD, 128, D).transpose(1, 0, 2)
    bias_pk = bias.reshape(KD, 128).T
    w2a_pk = w2a.astype(np.float16).reshape(KD, 128, 11).transpose(1, 0, 2)

    x16 = np.zeros((B, KXP), np.float16)
    x16[:, :KX] = x.astype(np.float16)

    shared = {
        "w_c": np.ascontiguousarray(wc_pk),
        "w_res": np.ascontiguousarray(wres_pk),
        "bias": np.ascontiguousarray(bias_pk),
        "w2a": np.ascontiguousarray(w2a_pk),
        "w1": np.ascontiguousarray(w1v),
        "b2": np.ascontiguousarray(b2v),
    }
    in_maps = []
    for c in range(N_CORES):
        m = dict(shared)
        # x.T packed as [p, half, k, r]: xt[p, h, k, r] = x[512h+r, 128k+p]
        xc = x16[c * R:(c + 1) * R, :].reshape(NS, 512, KE, 128)
        m["x"] = np.ascontiguousarray(xc.transpose(3, 0, 2, 1))
        in_maps.append(m)
    return in_maps


def run(inputs, trace=False, tmpdir=None):
    """Run on 8 NeuronCores; returns (out [8192,10], BassKernelResults)."""
    nc = _get_program()
    in_maps = _prepare_in_maps(inputs)
    res = bass_utils.run_bass_kernel_spmd(
        nc, in_maps, core_ids=list(range(N_CORES)), trace=trace, tmpdir=tmpdir)
    outs = [np.asarray(r["out"]).astype(np.float32).reshape(128, KD, 10)
            .transpose(1, 0, 2).reshape(R, 10) for r in res.results]
    return np.concatenate(outs, axis=0), res


def kernel(**inputs):
    out, _ = run(inputs, trace=False)
    return out
